# revision 14
# baseline (speedup 1.0000x reference)
"""Mamba-core (4-layer) Trainium2 Bass kernel, v2.

Sharding: data-parallel over batch B=8 across 8 NeuronCores (one sample per
core, zero collectives).  Per core, activations live in SBUF in
[feature, time] layout, bf16 end-to-end (the scan's internal state stays
fp32 inside the tensor_tensor_scan instruction).

Differences vs v1 (98 ms → target):
  - every matmul operand is bf16 (4x fewer PE cycles/row than fp32)
  - silu(z) / silu(conv+b) are single ACT-table ops (Silu table)
  - softplus = Exp then Ln(1+x); both live in one ACT table set together
    with the scan's Exp, so only 2 table loads per layer
  - scan-stage tiles are [128, 1024] (2x wider than v1): quarter-outer,
    n-inner loop, state chained across quarters via hlast
  - B/C row replication matmuls write PSUM, then one copy per tile to
    SBUF bf16 so the bt/tmp multiplies run in the DVE 2x (16-bit) mode
  - work is spread across four engines: PE (matmuls), ACT (activations +
    some rep copies), DVE (bt/tmp/dtu/hlast), Pool/GpSimd (scans + most
    rep copies + gating)
  - the D*xa skip term is folded into the y PSUM accumulation as one
    extra diag-matmul per quarter
"""

import os
import numpy as np

DM = 128        # d_model
DI = 256        # d_inner
NDH = 2         # d_inner halves of 128
NST = 16        # d_state
RNK = 8         # dt_rank
L = 4096
LAYERS = 4
DCONV = 4
CH = 512        # matmul moving-dim chunk (PSUM bank)
Q = 1024        # scan-stage quarter (one psacc tile)
NQ = L // Q     # 4
B = 8
NCORES = 8

USE_SILU = True  # Silu ACT table on HW; False falls back to Sigmoid+mult

# blob column offsets (per layer): wz 256 | wxa 1024 | wxp 192 | wo 256 | wd 256 | wdt 256
LW = DI + DCONV * DI + NDH * 96 + NDH * DM + NDH * DM + DI   # 2240
XTC = L + 3  # xT columns, prepended to the per-core blob
BLOBC = LAYERS * LW + DM + NST * DM                          # + ident 128 + sel 2048


def prep_weights(inputs):
    """Host-side weight preprocessing (numpy, tiny)."""
    import ml_dtypes
    bf = ml_dtypes.bfloat16

    in_w = inputs["in_proj_w"]    # [4, 512, 128]
    cw = inputs["conv_w"]         # [4, 256, 4]
    cb = inputs["conv_b"]         # [4, 256]
    xp_w = inputs["x_proj_w"]     # [4, 40, 256]
    dtp_w = inputs["dt_proj_w"]   # [4, 256, 8]
    dtp_b = inputs["dt_proj_b"]   # [4, 256]
    Dp = inputs["D"]              # [4, 256]
    out_w = inputs["out_proj_w"]  # [4, 128, 256]

    wz = np.ascontiguousarray(np.transpose(in_w[:, DI:, :], (0, 2, 1)))  # [4,128,256]
    # conv folded into in_proj: wxa[l, kd, k*DI+m] = cw[l, m, k] * in_w[l, m, kd]
    wxa = np.einsum("lmk,lmd->ldkm", cw, in_w[:, :DI, :])                # [4,128,4,256]
    wxa = np.ascontiguousarray(wxa.reshape(LAYERS, DM, DCONV * DI))
    # wxp[l, ksub, dh*96 + seg]: x_proj output padded to M=96 so the PSUM
    # splits land on 32-aligned partitions: dtraw @ 0:8, Bm @ 32:48, Cm @ 64:80
    wxp_t = np.transpose(xp_w.reshape(LAYERS, 40, NDH, DM), (0, 3, 2, 1))  # [l,ksub,dh,40]
    wxp = np.zeros((LAYERS, DM, NDH, 96), np.float32)
    wxp[:, :, :, 0:RNK] = wxp_t[:, :, :, 0:RNK]
    wxp[:, :, :, 32:32 + NST] = wxp_t[:, :, :, RNK:RNK + NST]
    wxp[:, :, :, 64:64 + NST] = wxp_t[:, :, :, RNK + NST:RNK + 2 * NST]
    wxp = np.ascontiguousarray(wxp.reshape(LAYERS, DM, NDH * 96))
    wdt = np.ascontiguousarray(np.transpose(dtp_w, (0, 2, 1)))           # [4,8,256]
    # wo[l, ksub, dh*128+m] = out_w[l, m, dh*128+ksub]
    wo = np.transpose(out_w.reshape(LAYERS, DM, NDH, DM), (0, 3, 2, 1))
    wo = np.ascontiguousarray(wo.reshape(LAYERS, DM, NDH * DM))
    # wd: diag(D) per half — folds the D*xa skip into the y PSUM accumulation
    wd = np.zeros((LAYERS, DM, NDH * DM), np.float32)
    for l in range(LAYERS):
        for dh in range(NDH):
            np.fill_diagonal(wd[l, :, dh * DM:(dh + 1) * DM], Dp[l, dh * DM:(dh + 1) * DM])
    vecs = np.zeros((LAYERS, DM, 6), np.float32)
    for dh in range(NDH):
        s = slice(dh * DM, (dh + 1) * DM)
        vecs[:, :, 0 + dh] = cb[:, s]
        vecs[:, :, 2 + dh] = dtp_b[:, s]
        vecs[:, :, 4 + dh] = Dp[:, s]
    # selp[32+k or 64+k, n*128+p] = 1 iff k == n — row-n replicator lhsT,
    # placed at partition bases 32 and 64 so lhsT base matches the rhs base
    # (Bm rows live at pjs[32:48], Cm rows at pjs[64:80]).
    sel = np.zeros((80, NST * DM), np.float32)
    for n in range(NST):
        sel[32 + n, n * DM:(n + 1) * DM] = 1.0
        sel[64 + n, n * DM:(n + 1) * DM] = 1.0
    # pack every bf16 weight into one [128, BLOBC] blob (fewer per-exec args)
    def pad128(a):
        out = np.zeros((128,) + a.shape[1:], a.dtype)
        out[:a.shape[0]] = a
        return out

    per_layer = []
    for l in range(LAYERS):
        per_layer.append(np.concatenate([
            wz[l], wxa[l], wxp[l], wo[l], wd[l], pad128(wdt[l])], axis=1))
    blob = np.concatenate(per_layer + [np.eye(DM, dtype=np.float32),
                                       pad128(sel)], axis=1)
    assert blob.shape[1] == BLOBC, blob.shape
    return {
        "blob": blob.astype(bf),
        "vecs": vecs.astype(np.float32),
    }


def make_in_maps(inputs):
    """Per-core input dicts (shared by kernel() and test harnesses)."""
    import ml_dtypes
    w = prep_weights(inputs)
    x = inputs["x"]
    in_maps = []
    for bb in range(NCORES):
        blob = np.zeros((DM, XTC + BLOBC), ml_dtypes.bfloat16)
        blob[:, 3:XTC] = x[bb].T.astype(ml_dtypes.bfloat16)
        blob[:, XTC:] = w["blob"]
        in_maps.append({"blob": blob, "vecs": w["vecs"]})
    return in_maps


def build_program(layers=LAYERS):
    import concourse.tile as tile
    from concourse import bacc, mybir
    from contextlib import ExitStack

    f32 = mybir.dt.float32
    bf16 = mybir.dt.bfloat16
    AF = mybir.ActivationFunctionType
    OP = mybir.AluOpType

    nc = bacc.Bacc("TRN2")

    blob_d = nc.dram_tensor("blob", [DM, XTC + BLOBC], bf16, kind="ExternalInput")
    vecs_d = nc.dram_tensor("vecs", [LAYERS, DM, 6], f32, kind="ExternalInput")
    out_d = nc.dram_tensor("out", [DM, L], f32, kind="ExternalOutput")

    with tile.TileContext(nc) as tc, ExitStack() as ctx:
        pers = ctx.enter_context(tc.tile_pool(name="pers", bufs=1))
        wts = ctx.enter_context(tc.tile_pool(name="wts", bufs=2))
        work = ctx.enter_context(tc.tile_pool(name="work", bufs=3))
        cold = ctx.enter_context(tc.tile_pool(name="cold", bufs=2))
        ps = ctx.enter_context(tc.tile_pool(name="ps", bufs=2, space="PSUM"))
        psacc = ctx.enter_context(tc.tile_pool(name="psacc", bufs=1, space="PSUM"))

        xt = pers.tile([DM, L + 3], bf16, tag="xt", name="xt")
        nc.sync.dma_start(xt[:], blob_d[:, 0:XTC])
        wb = XTC
        ident = pers.tile([DM, DM], bf16, tag="ident", name="ident")
        nc.sync.dma_start(ident[:], blob_d[:, wb + LAYERS * LW:wb + LAYERS * LW + DM])
        sel = pers.tile([80, NST * DM], bf16, tag="sel", name="sel")
        nc.sync.dma_start(sel[:], blob_d[0:80, wb + LAYERS * LW + DM:wb + LAYERS * LW + DM + NST * DM])

        xa = [pers.tile([DM, L], bf16, tag=f"xa{dh}", name=f"xa{dh}") for dh in range(NDH)]
        dts = [pers.tile([DM, L], bf16, tag=f"dt{dh}", name=f"dt{dh}") for dh in range(NDH)]
        dtu = [pers.tile([DM, L], bf16, tag=f"dtu{dh}", name=f"dtu{dh}") for dh in range(NDH)]
        sz = [pers.tile([DM, L], bf16, tag=f"sz{dh}", name=f"sz{dh}") for dh in range(NDH)]
        pjs = pers.tile([96, L], bf16, tag="pjs", name="pjs")
        hlast = pers.tile([DM, NDH * NST], f32, tag="hlast", name="hlast")

        copy_ct = 0  # round-robins the rep copies between ACT and Pool

        for layer in range(layers):
            wl = layer % LAYERS
            o = XTC + wl * LW
            w_z = wts.tile([DM, DI], bf16, tag="w_z", name="w_z")
            nc.sync.dma_start(w_z[:], blob_d[:, o:o + DI])
            o += DI
            w_xa = wts.tile([DM, DCONV * DI], bf16, tag="w_xa", name="w_xa")
            nc.sync.dma_start(w_xa[:], blob_d[:, o:o + DCONV * DI])
            o += DCONV * DI
            w_xp = wts.tile([DM, NDH * 96], bf16, tag="w_xp", name="w_xp")
            nc.sync.dma_start(w_xp[:], blob_d[:, o:o + NDH * 96])
            o += NDH * 96
            w_o = wts.tile([DM, NDH * DM], bf16, tag="w_o", name="w_o")
            nc.sync.dma_start(w_o[:], blob_d[:, o:o + NDH * DM])
            o += NDH * DM
            w_d = wts.tile([DM, NDH * DM], bf16, tag="w_d", name="w_d")
            nc.sync.dma_start(w_d[:], blob_d[:, o:o + NDH * DM])
            o += NDH * DM
            w_dt = wts.tile([RNK, DI], bf16, tag="w_dt", name="w_dt")
            nc.sync.dma_start(w_dt[:], blob_d[0:RNK, o:o + DI])
            vec = wts.tile([DM, 6], f32, tag="vec", name="vec")
            nc.sync.dma_start(vec[:], vecs_d[wl])

            # ---- stage A1: in_proj+conv, silu gates, x_proj (Silu table) ----
            for q in range(NQ):
                t0 = q * Q
                for dh in range(NDH):
                    mslc = slice(dh * DM, (dh + 1) * DM)
                    p_z = ps.tile([DM, Q], f32, tag="rep", name="rep")
                    for c in range(Q // CH):
                        cs = slice(c * CH, (c + 1) * CH)
                        nc.tensor.matmul(p_z[:, cs], w_z[:, mslc],
                                         xt[:, 3 + t0 + c * CH:3 + t0 + (c + 1) * CH],
                                         start=True, stop=True)
                    if USE_SILU:
                        nc.scalar.activation(sz[dh][:, t0:t0 + Q], p_z[:], AF.Silu)
                    else:
                        sg = cold.tile([DM, Q], bf16, tag="sg", name="sg")
                        nc.scalar.activation(sg[:], p_z[:], AF.Sigmoid)
                        nc.vector.tensor_tensor(sz[dh][:, t0:t0 + Q], p_z[:], sg[:],
                                                OP.mult)
                    p_xa = ps.tile([DM, Q], f32, tag="rep", name="rep")
                    for c in range(Q // CH):
                        cs = slice(c * CH, (c + 1) * CH)
                        for k in range(DCONV):
                            nc.tensor.matmul(
                                p_xa[:, cs],
                                w_xa[:, k * DI + dh * DM:k * DI + (dh + 1) * DM],
                                xt[:, t0 + c * CH + k:t0 + (c + 1) * CH + k],
                                start=(k == 0), stop=(k == DCONV - 1))
                    if USE_SILU:
                        nc.scalar.activation(xa[dh][:, t0:t0 + Q], p_xa[:], AF.Silu,
                                             bias=vec[:, 0 + dh:1 + dh])
                    else:
                        sgx = cold.tile([DM, Q], bf16, tag="sg", name="sg")
                        nc.scalar.activation(sgx[:], p_xa[:], AF.Sigmoid,
                                             bias=vec[:, 0 + dh:1 + dh])
                        ux = cold.tile([DM, Q], f32, tag="ux", name="ux")
                        nc.scalar.activation(ux[:], p_xa[:], AF.Identity,
                                             bias=vec[:, 0 + dh:1 + dh])
                        nc.vector.tensor_tensor(xa[dh][:, t0:t0 + Q], ux[:], sgx[:],
                                                OP.mult)
                p_pj = ps.tile([96, Q], f32, tag="rep", name="rep")
                for c in range(Q // CH):
                    cs = slice(c * CH, (c + 1) * CH)
                    for dh in range(NDH):
                        nc.tensor.matmul(p_pj[:, cs], w_xp[:, dh * 96:(dh + 1) * 96],
                                         xa[dh][:, t0 + c * CH:t0 + (c + 1) * CH],
                                         start=(dh == 0), stop=(dh == NDH - 1))
                nc.scalar.copy(pjs[:, t0:t0 + Q], p_pj[:])
            # ---- stage A2: dt = softplus via Exp then Ln (exp/ln table) ----
            for q in range(NQ):
                t0 = q * Q
                for dh in range(NDH):
                    mslc = slice(dh * DM, (dh + 1) * DM)
                    p_dt = ps.tile([DM, Q], f32, tag="rep", name="rep")
                    for c in range(Q // CH):
                        cs = slice(c * CH, (c + 1) * CH)
                        nc.tensor.matmul(p_dt[:, cs], w_dt[:, mslc],
                                         pjs[0:RNK, t0 + c * CH:t0 + (c + 1) * CH],
                                         start=True, stop=True)
                    nc.scalar.activation(dts[dh][:, t0:t0 + Q], p_dt[:], AF.Exp,
                                         bias=vec[:, 2 + dh:3 + dh])
            for dh in range(NDH):
                nc.scalar.activation(dts[dh][:], dts[dh][:], AF.Ln, bias=1.0)
                nc.vector.tensor_tensor(dtu[dh][:], dts[dh][:], xa[dh][:], OP.mult)

            # ---- scan stage: quarter-outer, n-inner ----
            for q in range(NQ):
                t0 = q * Q
                acc = [psacc.tile([DM, Q], f32, tag=f"acc{dh}", name=f"acc{dh}")
                       for dh in range(NDH)]
                # n-order pairs k with k+8 so a_{k+8} = a_k * a_7 needs only
                # the just-computed a_k plus the persistent a_7 (one extra
                # bf16 rounding on the n>=8 decay factors)
                a7 = [None, None]
                aprev = [None, None]
                for ni, n in enumerate([7, 0, 8, 1, 9, 2, 10, 3, 11, 4, 12, 5, 13, 6, 14, 15]):
                    p_b = ps.tile([DM, Q], f32, tag="rep", name="rep")
                    for c in range(Q // CH):
                        cs = slice(c * CH, (c + 1) * CH)
                        nc.tensor.matmul(p_b[:, cs],
                                         sel[32:32 + NST, n * DM:(n + 1) * DM],
                                         pjs[32:32 + NST, t0 + c * CH:t0 + (c + 1) * CH],
                                         start=True, stop=True)
                    brep = work.tile([DM, Q], bf16, tag="brep", name="brep")
                    if copy_ct % 6 < 5:
                        nc.scalar.copy(brep[:], p_b[:])
                    else:
                        nc.vector.tensor_copy(brep[:], p_b[:])
                    copy_ct += 1
                    p_c = ps.tile([DM, Q], f32, tag="rep", name="rep")
                    for c in range(Q // CH):
                        cs = slice(c * CH, (c + 1) * CH)
                        nc.tensor.matmul(p_c[:, cs],
                                         sel[64:64 + NST, n * DM:(n + 1) * DM],
                                         pjs[64:64 + NST, t0 + c * CH:t0 + (c + 1) * CH],
                                         start=True, stop=True)
                    crep = work.tile([DM, Q], bf16, tag="crep", name="crep")
                    if copy_ct % 6 < 5:
                        nc.scalar.copy(crep[:], p_c[:])
                    else:
                        nc.vector.tensor_copy(crep[:], p_c[:])
                    copy_ct += 1
                    for dh in range(NDH):
                        if n == 7:
                            at = work.tile([DM, Q], bf16, tag=f"a7_{dh}", name=f"a7_{dh}")
                            nc.scalar.activation(at[:], dts[dh][:, t0:t0 + Q],
                                                 AF.Exp, scale=-8.0)
                            a7[dh] = at
                        elif n < 8:
                            at = work.tile([DM, Q], bf16, tag=f"at{dh}", name=f"at{dh}")
                            nc.scalar.activation(at[:], dts[dh][:, t0:t0 + Q],
                                                 AF.Exp, scale=-float(n + 1))
                            aprev[dh] = at
                        else:
                            src = a7[dh] if n == 15 else aprev[dh]
                            at = work.tile([DM, Q], bf16, tag=f"at{dh}", name=f"at{dh}")
                            nc.gpsimd.tensor_tensor(at[:], src[:], a7[dh][:], OP.mult)
                        btl = work.tile([DM, Q], bf16, tag="bt", name="bt")
                        bt_eng = nc.gpsimd if (ni * 2 + dh) % 4 < 3 else nc.vector
                        bt_eng.tensor_tensor(btl[:], dtu[dh][:, t0:t0 + Q],
                                             brep[:], OP.mult)
                        ht = work.tile([DM, Q], bf16, tag=f"ht{dh}", name=f"ht{dh}")
                        init = hlast[:, dh * NST + n:dh * NST + n + 1] if q > 0 else 0.0
                        nc.vector.tensor_tensor_scan(ht[:], at[:], btl[:], init,
                                                      OP.mult, OP.add)
                        tmp = work.tile([DM, Q], bf16, tag="tmp", name="tmp")
                        tmp_eng = nc.gpsimd if (ni * 2 + dh) % 4 < 3 else nc.vector
                        tmp_eng.tensor_tensor(tmp[:], ht[:], crep[:], OP.mult)
                        for c in range(Q // CH):
                            cs = slice(c * CH, (c + 1) * CH)
                            nc.tensor.matmul(acc[dh][:, cs], ident[:], tmp[:, cs],
                                             start=(ni == 0), stop=False)
                        if q < NQ - 1:
                            nc.vector.tensor_copy(
                                hlast[:, dh * NST + n:dh * NST + n + 1],
                                ht[:, Q - 1:Q])
                # stage C: fold D*xa into the accumulation, gate, out_proj
                ygs = []
                for dh in range(NDH):
                    for c in range(Q // CH):
                        cs = slice(c * CH, (c + 1) * CH)
                        nc.tensor.matmul(acc[dh][:, cs], w_d[:, dh * DM:(dh + 1) * DM],
                                         xa[dh][:, t0 + c * CH:t0 + (c + 1) * CH],
                                         start=False, stop=True)
                    yg = cold.tile([DM, Q], bf16, tag=f"yg{dh}", name=f"yg{dh}")
                    nc.vector.tensor_tensor(yg[:], acc[dh][:], sz[dh][:, t0:t0 + Q],
                                            OP.mult)
                    ygs.append(yg)
                p_x = ps.tile([DM, Q], f32, tag="rep", name="rep")
                for c in range(Q // CH):
                    cs = slice(c * CH, (c + 1) * CH)
                    for dh in range(NDH):
                        nc.tensor.matmul(p_x[:, cs], w_o[:, dh * DM:(dh + 1) * DM],
                                         ygs[dh][:, cs], start=(dh == 0),
                                         stop=(dh == NDH - 1))
                if layer < layers - 1:
                    nc.scalar.copy(xt[:, 3 + t0:3 + t0 + Q], p_x[:])
                else:
                    ot = cold.tile([DM, Q], f32, tag="ot", name="ot")
                    nc.scalar.copy(ot[:], p_x[:])
                    nc.sync.dma_start(out_d[:, t0:t0 + Q], ot[:])
    nc.compile()
    return nc


_last_results = None


def kernel(**inputs):
    global _last_results
    from concourse.bass_utils import run_bass_kernel_spmd

    in_maps = make_in_maps(inputs)
    nc = build_program()
    # the axon NTFF hook is absent in this container; never trace here
    os.environ["BASS_NEVER_TRACE"] = "1"
    br = run_bass_kernel_spmd(nc, in_maps, core_ids=list(range(NCORES)),
                              trace=False)
    _last_results = br
    out = np.empty((B, L, DM), np.float32)
    for bb in range(NCORES):
        out[bb] = br.results[bb]["out"].T
    return out
